# revision 1
# baseline (speedup 1.0000x reference)
"""Trainium2 Bass kernel for MultiHeadSelfAttention (K-only variant).

Math (per batch b):
    K  = x @ Wk.T;  Kh = heads(K)
    S_h = Kh @ Kh.T / sqrt(D);  P_h = softmax(S_h)
    wV_h = P_h @ Kh  (V == K);  out = concat_h(wV) @ Wo.T

Sharding (8 cores): core c handles batch c//2 and query-half c%2 with all
heads.  The query half is selected by rolling x on the host so each core
always computes queries 0:S//2 of its (rolled) sequence; softmax over keys
is order-invariant so rolling the key axis is harmless.

Per-core pipeline (one SPMD NEFF):
    xT_bf  = XBAR-DMA-transpose(bf16(x))     [d, s]
    K      = xT_bf.T @ WkT_bf  (bf16 matmuls, fp32 psum) -> k_bf, kones
    khT    = XBAR-DMA-transpose(k_bf)        [e, s] bf16
    per (qb, head):
      scores strip S_T[k, q] = khT_h.T @ khT_h[:, qb]    (bf16)
      E_T = exp(S_T / sqrt(D))       (ScalarE, psum -> sbuf bf16)
      PV:  [wVT_h ; rowsum_h] = [Kh_h | 1].T @ E_T       (bf16, psum accum)
      recip = 1/rowsum; partition-broadcast via K=1 matmul; normalize wVT
    out = wVTn.T @ WoT  (fp32r), first half overlapped with second qb
"""

import sys

if "/opt/trn_rl_repo" not in sys.path:
    sys.path.insert(0, "/opt/trn_rl_repo")

import numpy as np

B, S, D = 4, 2048, 512
H = 8
HD = D // H            # 64
P = 128
SH = S // 2            # query half per core = 1024
NCORES = 8
SCALE = 1.0 / np.sqrt(D)

_CACHE = {}


def _build_nc(repeat: int = 1, mode: str = "full"):
    import concourse.bass as bass  # noqa: F401
    import concourse.tile as tile
    import concourse.mybir as mybir
    from concourse import bacc
    from concourse.masks import make_identity
    from contextlib import ExitStack

    f32 = mybir.dt.float32
    f32r = mybir.dt.float32r
    bf16 = mybir.dt.bfloat16

    nc = bacc.Bacc("TRN2", target_bir_lowering=False, debug=False,
                   num_devices=NCORES)

    x_d = nc.dram_tensor("x", [S, D], f32, kind="ExternalInput").ap()
    wk_d = nc.dram_tensor("Wk", [D, D], f32, kind="ExternalInput").ap()
    wo_d = nc.dram_tensor("Wo", [D, D], f32, kind="ExternalInput").ap()
    out_d = nc.dram_tensor("out", [SH, D], f32, kind="ExternalOutput").ap()

    NSC = S // P           # 16 sequence chunks
    NDC = D // P           # 4 feature chunks
    NQB = SH // 512        # 2 query blocks of 512
    QB = 512

    import contextlib
    with tile.TileContext(nc) as tc:
        loop_cm = tc.For_i(0, repeat, 1) if repeat > 1 else contextlib.nullcontext()
        with loop_cm, ExitStack() as ctx:
            consts = ctx.enter_context(tc.tile_pool(name="consts", bufs=1))
            kpool = ctx.enter_context(tc.tile_pool(name="kpool", bufs=1))
            _ebufs = 5 if "ct" in mode else (3 if "e3" in mode else 4)
            epool = ctx.enter_context(
                tc.tile_pool(name="epool", bufs=_ebufs))
            vpool = ctx.enter_context(tc.tile_pool(name="vpool", bufs=1))
            opool = ctx.enter_context(
                tc.tile_pool(name="opool", bufs=(3 if "x4" in mode else 2)))
            # psum: tag A = 4 banks x1, tag B = 2 banks x2  -> 8 banks total
            ps = ctx.enter_context(tc.tile_pool(name="ps", bufs=1, space="PSUM"))

            bigspan = "bigspan" in mode

            def spanA(dtype=None):
                return ps.tile([P, 4, 512], dtype or f32, tag="A",
                               bufs=(2 if bigspan else 1), name="spA")

            def spanB(shape=None, name="spB"):
                if bigspan:
                    return ps.tile(shape or [P, 2, 512], f32, tag="A", bufs=2,
                                   name=name)
                return ps.tile(shape or [P, 2, 512], f32, tag="B", bufs=2,
                               name=name)

            ident = consts.tile([P, P], f32)
            make_identity(nc, ident[:])

            ones1x64f = consts.tile([1, 64], f32)
            nc.gpsimd.memset(ones1x64f[:], 1.0)
            ones1x64 = consts.tile([1, 64], f32r)
            nc.vector.tensor_copy(ones1x64[:], ones1x64f[:])
            ones_bf = consts.tile([P, 1], bf16)
            nc.gpsimd.memset(ones_bf[:], 1.0)
            ident_bf = consts.tile([P, P], bf16)
            nc.vector.tensor_copy(ident_bf[:], ident[:])

            woT = consts.tile([P, NDC, 512], f32r)      # [d', e]
            k_bf = kpool.tile([P, NSC, 512], bf16)      # K [s, e]
            kones = (None if "ct" in mode else
                     kpool.tile([P, NSC, H, HD + 1], bf16))
            khT = kpool.tile([P, NDC, S], bf16)         # K^T [e, s]
            wvt = vpool.tile([P, NDC, SH], f32r)        # wVT (norm in place)

            # ---- phase 0/1: weights, x -> xT (PE transposes) -> K proj -----
            with tc.tile_pool(name="stage", bufs=1) as stage:
                wk_sb = stage.tile([P, NDC, 512], f32, tag="w")
                nc.sync.dma_start(wk_sb[:], wk_d.rearrange("(eo p) d -> p eo d", p=P))
                wkT_r = consts.tile([P, NDC, 512], f32r)
                sp = spanA()
                for dc in range(NDC):
                    for eo in range(NDC):
                        nc.tensor.transpose(
                            sp[:, dc, eo * P:(eo + 1) * P],
                            wk_sb[:, eo, dc * P:(dc + 1) * P], ident[:])
                nc.scalar.copy(wkT_r[:], sp[:])

                # x: 8 groups of 2 seq-chunks; PE transpose -> fp32r Kproj
                for g in range(8):
                    g0 = g * 2
                    x_g = stage.tile([P, 2, 512], f32, tag="x",
                                     bufs=(4 if "x4" in mode else 3),
                                     name="x_g")
                    nc.sync.dma_start(
                        x_g[:, 0:2, :],
                        x_d[g0 * P:(g0 + 2) * P, :].rearrange(
                            "(two p) d -> p two d", p=P))
                    spt = spanB([P, 4, 256], name="sptr")
                    for i in range(2):
                        for dc in range(NDC):
                            nc.tensor.transpose(
                                spt[:, dc, i * P:(i + 1) * P],
                                x_g[:, i, dc * P:(dc + 1) * P], ident[:])
                    xT_g = stage.tile([P, NDC, 256], f32r, tag="xT", bufs=2,
                                      name="xT_g")
                    nc.scalar.copy(xT_g[:], spt[:])

                    spk = spanB(name="spkp")
                    for i in range(2):
                        for dc in range(NDC):
                            nc.tensor.matmul(
                                spk[:, i, :],
                                xT_g[:, dc, i * P:(i + 1) * P],
                                wkT_r[:, dc, :],
                                start=(dc == 0), stop=(dc == NDC - 1))
                    nc.vector.tensor_copy(k_bf[:, g0:g0 + 2, :], spk[:, 0:2, :])
                    if kones is not None:
                        nc.vector.tensor_copy(
                            kones[:, g0:g0 + 2, :, 0:HD],
                            spk[:, 0:2, :].rearrange("p g (h e) -> p g h e",
                                                     h=H))
                    if "pekt" in mode:
                        # khT via PE transposes (bf16): 8 tiles -> one A span
                        spkt = spanA(bf16)
                        for i in range(2):
                            sc = g0 + i
                            for ec in range(NDC):
                                nc.tensor.transpose(
                                    spkt[:, ec, i * P:(i + 1) * P],
                                    k_bf[:, sc, ec * P:(ec + 1) * P],
                                    ident_bf[:])
                        nc.vector.tensor_copy(
                            khT[:, :, g0 * P:(g0 + 2) * P],
                            spkt[:, 0:NDC, 0:2 * P])

                # Wo DMA early (keeps all DMACopies before the XBAR
                # cluster); its PE transposes run after the khT transposes
                # so head-0 scores can start as soon as khT chunks land.
                wo_sb = stage.tile([P, NDC, 512], f32, tag="w")
                nc.sync.dma_start(wo_sb[:], wo_d.rearrange("(eo p) d -> p eo d", p=P))

                if "pekt" not in mode:
                    # khT via XBAR transposes, one cluster after all DMAs
                    for sc in range(NSC):
                        nc.sync.dma_start_transpose(
                            khT[:, :, sc * P:(sc + 1) * P], k_bf[:, sc, :])

                # Wo: transpose via PE (fp32 -> fp32r), span A
                sp = spanA()
                for dc in range(NDC):
                    for eo in range(NDC):
                        nc.tensor.transpose(
                            sp[:, dc, eo * P:(eo + 1) * P],
                            wo_sb[:, eo, dc * P:(dc + 1) * P], ident[:])
                nc.scalar.copy(woT[:], sp[:])

            if kones is not None:
                nc.gpsimd.memset(kones[:, :, :, HD:HD + 1], 1.0)

            if "phase0" in mode:
                # consume everything so DCE cannot strip phase 0/1 work
                with tc.tile_pool(name="sink", bufs=1, space="DRAM") as sink:
                    snk1 = sink.tile([P, NSC, 512], bf16, name="snk1")
                    nc.sync.dma_start(snk1[:], k_bf[:])
                    snk2 = sink.tile([P, NDC, S], bf16, name="snk2")
                    nc.sync.dma_start(snk2[:], khT[:])
                    snk3 = sink.tile([P, NSC, H, HD + 1], bf16, name="snk3")
                    nc.sync.dma_start(snk3[:], kones[:])
                    snk4 = sink.tile([P, NDC, 512], f32, name="snk4")
                    nc.sync.dma_start(snk4[:], woT[:].bitcast(f32))
                o_sb0 = opool.tile([P, 2, 512], f32, tag="osb", name="o_sb0")
                nc.vector.tensor_copy(o_sb0[:, 0, :], khT[:, 0, 0:512])
                nc.vector.tensor_copy(o_sb0[:, 1, :], k_bf[:, 0, :])
                nc.sync.dma_start(
                    out_d[0:2 * P, :].rearrange("(two p) d -> p two d", p=P),
                    o_sb0[:])

            # ---- head loop (qb outer), software-pipelined PV ----------------
            if bigspan:
                kc_groups = [(0, "A"), (4, "A"), (8, "A"), (12, "A")]
            else:
                kc_groups = [(0, "A"), (4, "B"), (6, "B"), (8, "A"),
                             (12, "B"), (14, "B")]

            def emit_pv(h, qb, e_t):
                hp = (h % 2) * HD
                ec = h // 2
                pv = spanB([HD + 1, 512], name="pv")
                for kc in range(NSC):
                    nc.tensor.matmul(
                        pv[:], kones[:, kc, h, :], e_t[:, kc, :],
                        start=(kc == 0), stop=(kc == NSC - 1))
                nc.vector.tensor_copy(
                    wvt[hp:hp + HD, ec, qb * QB:(qb + 1) * QB], pv[0:HD, :])
                recip_t = vpool.tile([1, 512], f32r, tag="recip", bufs=4,
                                     name="recip_t")
                with nc.allow_low_precision(reason="fp32r recip is fine"):
                    nc.vector.reciprocal(recip_t[:], pv[HD:HD + 1, :])
                bc = spanB([HD, 512], name="bc")
                nc.tensor.matmul(
                    bc[:], ones1x64[:], recip_t[:], start=True, stop=True)
                nc.vector.tensor_tensor(
                    wvt[hp:hp + HD, ec, qb * QB:(qb + 1) * QB],
                    wvt[hp:hp + HD, ec, qb * QB:(qb + 1) * QB],
                    bc[:], mybir.AluOpType.mult)

            def emit_outproj(qc0):
                # two q-chunks of 128 per pass, psum in a B slot
                po = spanB(name="po")
                for j in range(2):
                    qc = qc0 + j
                    for dc in range(NDC):
                        nc.tensor.matmul(
                            po[:, j, :],
                            wvt[:, dc, qc * P:(qc + 1) * P],
                            woT[:, dc, :],
                            start=(dc == 0), stop=(dc == NDC - 1))
                o_sb = opool.tile([P, 2, 512], f32, tag="osb", name="o_sb")
                nc.vector.tensor_copy(o_sb[:], po[:])
                nc.sync.dma_start(
                    out_d[qc0 * P:(qc0 + 2) * P, :].rearrange(
                        "(two p) d -> p two d", p=P),
                    o_sb[:])

            def emit_pv_pair(j, qb, e_lo, e_hi):
                # heads (2j, 2j+1) concurrently via PE column tiling
                pv = spanB([P, 512], name="pvp")
                for kc in range(NSC):
                    nc.tensor.matmul(
                        pv[0:HD, :],
                        k_bf[:, kc, (2 * j) * HD:(2 * j + 1) * HD],
                        e_lo[:, kc, :],
                        start=(kc == 0), stop=(kc == NSC - 1),
                        tile_position=(0, 0))
                    nc.tensor.matmul(
                        pv[HD:2 * HD, :],
                        k_bf[:, kc, (2 * j + 1) * HD:(2 * j + 2) * HD],
                        e_hi[:, kc, :],
                        start=(kc == 0), stop=(kc == NSC - 1),
                        tile_position=(0, HD))
                nc.vector.tensor_copy(
                    wvt[:, j, qb * QB:(qb + 1) * QB], pv[:])

            def emit_rs_quad(g, qb, e_ts):
                # rowsums of heads 4g..4g+3 via 4-way column tiling (M=1)
                rs = spanB([97, 512], name="rs")
                for kc in range(NSC):
                    for hi in range(4):
                        nc.tensor.matmul(
                            rs[32 * hi:32 * hi + 1, :],
                            ones_bf[:, 0:1],
                            e_ts[hi][:, kc, :],
                            start=(kc == 0), stop=(kc == NSC - 1),
                            tile_position=(0, 32 * hi))
                for hi in range(4):
                    h = 4 * g + hi
                    hp = (h % 2) * HD
                    ec = h // 2
                    recip_t = vpool.tile([1, 512], f32r, tag="recip", bufs=4,
                                         name="recip_t")
                    with nc.allow_low_precision(reason="fp32r recip is fine"):
                        nc.vector.reciprocal(recip_t[:],
                                             rs[32 * hi:32 * hi + 1, :])
                    bc = spanB([HD, 512], name="bc")
                    nc.tensor.matmul(
                        bc[:], ones1x64[:], recip_t[:], start=True, stop=True)
                    nc.vector.tensor_tensor(
                        wvt[hp:hp + HD, ec, qb * QB:(qb + 1) * QB],
                        wvt[hp:hp + HD, ec, qb * QB:(qb + 1) * QB],
                        bc[:], mybir.AluOpType.mult)

            if "ct" in mode:
                for qb in range(NQB):
                    quad = []
                    for h in range(H):
                        hp = (h % 2) * HD
                        ec = h // 2
                        e_t = epool.tile([P, NSC, 512], bf16, tag="E",
                                         name="e_t")
                        for g0, kind in kc_groups:
                            gn = 4 if kind == "A" else 2
                            sp = spanA() if kind == "A" else spanB()
                            for i in range(gn):
                                kc = g0 + i
                                nc.tensor.matmul(
                                    sp[:, i, :],
                                    khT[hp:hp + HD, ec, kc * P:(kc + 1) * P],
                                    khT[hp:hp + HD, ec, qb * QB:(qb + 1) * QB],
                                    start=True, stop=True)
                            nc.scalar.activation(
                                e_t[:, g0:g0 + gn, :], sp[:, 0:gn, :],
                                mybir.ActivationFunctionType.Exp, scale=SCALE)
                        quad.append(e_t)
                        if h % 2 == 1:
                            emit_pv_pair(h // 2, qb, quad[-2], quad[-1])
                        if h % 4 == 3:
                            emit_rs_quad(h // 4, qb, quad)
                            quad = []
                    for qc0 in range(qb * 4, qb * 4 + 4, 2):
                        emit_outproj(qc0)
            else:
                pending = None
                backlog = []
                for qb in range(NQB if "phase0" not in mode else 0):
                    for h in range(H):
                        hp = (h % 2) * HD
                        ec = h // 2
                        e_t = epool.tile([P, NSC, 512], bf16, tag="E", name="e_t")
                        for g0, kind in kc_groups:
                            gn = 4 if kind == "A" else 2
                            sp = spanA() if kind == "A" else spanB()
                            for i in range(gn):
                                kc = g0 + i
                                nc.tensor.matmul(
                                    sp[:, i, :],
                                    khT[hp:hp + HD, ec, kc * P:(kc + 1) * P],
                                    khT[hp:hp + HD, ec, qb * QB:(qb + 1) * QB],
                                    start=True, stop=True)
                            nc.scalar.activation(
                                e_t[:, g0:g0 + gn, :], sp[:, 0:gn, :],
                                mybir.ActivationFunctionType.Exp, scale=SCALE)
                        if pending is not None and "full" in mode:
                            emit_pv(*pending)
                            if pending[0] == H - 1:
                                backlog.extend(
                                    range(pending[1] * 4,
                                          pending[1] * 4 + 4, 2))
                            if (backlog and "spread" in mode
                                    and pending[0] % 2 == 1):
                                emit_outproj(backlog.pop(0))
                            elif backlog and "spread" not in mode:
                                while backlog:
                                    emit_outproj(backlog.pop(0))
                        pending = (h, qb, e_t)
                if "full" in mode:
                    emit_pv(*pending)
                    backlog.extend(
                        range(pending[1] * 4, pending[1] * 4 + 4, 2))
                    for qc0 in backlog:
                        emit_outproj(qc0)

    nc.compile()
    return nc


def _get_nc(repeat: int = 1, mode: str = "full"):
    key = ("nc", repeat, mode)
    if key not in _CACHE:
        _CACHE[key] = _build_nc(repeat, mode)
    return _CACHE[key]


def kernel(x: np.ndarray, Wk: np.ndarray, Wo: np.ndarray, _trace=False):
    from concourse import bass_utils

    nc = _get_nc()
    x = np.asarray(x, dtype=np.float32)
    Wk = np.ascontiguousarray(np.asarray(Wk, dtype=np.float32))
    Wo = np.ascontiguousarray(np.asarray(Wo, dtype=np.float32))

    in_maps = []
    for c in range(NCORES):
        b, half = c // 2, c % 2
        xb = x[b]
        if half:
            xb = np.roll(xb, -SH, axis=0)
        in_maps.append({"x": np.ascontiguousarray(xb), "Wk": Wk, "Wo": Wo})

    res = bass_utils.run_bass_kernel_spmd(
        nc, in_maps, core_ids=list(range(NCORES)), trace=_trace)

    out = np.empty((B, S, D), dtype=np.float32)
    for c in range(NCORES):
        b, half = c // 2, c % 2
        out[b, half * SH:(half + 1) * SH] = res.results[c]["out"]
    if _trace:
        _CACHE["last_results"] = res
    return out



# revision 2
# speedup vs baseline: 1.0796x; 1.0796x over previous
"""Trainium2 Bass kernel v4 for MultiHeadSelfAttention (K-only variant).

Q-sharded SPMD across 8 cores: core c = (b = c//2, half = c%2) handles
batch b, ALL 8 heads, query half `half` (host rolls x by half*1024; the
equal roll of q and k axes preserves S = K K^T symmetry).

v4 vs v3: the 120us serial phase-0 is restructured.  K^T is computed
DIRECTLY per 128-dim feature chunk ec (lhsT = wkT stationary, 2 matmuls
per LDWEIGHTS), so head-pair pr only waits for kproj(ec=pr).  kproj of
chunk ec+1 is emitted inside pair ec's exp stream and hides under ACT.
k_bf (PV operand) is one XBAR transpose of each khT chunk.

Per core:
    S_h = K_h K_h.T / sqrt(D), q local 0:1024, all k; E = exp blocks for
    kc >= 4*qb, mirrored for kc < 4 via XBAR transposes (E symmetric).
    wV = (E / rowsum) @ K;  out = wV @ Wo.T locally.
"""

import sys

if "/opt/trn_rl_repo" not in sys.path:
    sys.path.insert(0, "/opt/trn_rl_repo")

import numpy as np

B, S, D = 4, 2048, 512
H = 8
HD = D // H            # 64
P = 128
NCORES = 8
SH = S // 2
NSC = S // P           # 16
NDC = D // P           # 4
NQB = 2
QB = 512
NPR = 4
SCALE = 1.0 / np.sqrt(D)

_CACHE = {}


def _build_nc(repeat: int = 1, mode: str = "v4"):
    import concourse.bass as bass  # noqa: F401
    import concourse.tile as tile
    import concourse.mybir as mybir
    from concourse import bacc
    from concourse.masks import make_identity
    from contextlib import ExitStack
    import contextlib

    f32 = mybir.dt.float32
    f32r = mybir.dt.float32r
    bf16 = mybir.dt.bfloat16

    nc = bacc.Bacc("TRN2", target_bir_lowering=False, debug=False,
                   num_devices=NCORES)

    x_d = nc.dram_tensor("x", [S, D], f32, kind="ExternalInput").ap()
    wk_d = nc.dram_tensor("Wk", [D, D], f32, kind="ExternalInput").ap()
    wo_d = nc.dram_tensor("Wo", [D, D], f32, kind="ExternalInput").ap()
    out_d = nc.dram_tensor("out", [SH, D], f32, kind="ExternalOutput").ap()

    with tile.TileContext(nc) as tc:
        loop_cm = tc.For_i(0, repeat, 1) if repeat > 1 else contextlib.nullcontext()
        with loop_cm, ExitStack() as ctx:
            consts = ctx.enter_context(tc.tile_pool(name="consts", bufs=1))
            wpool = ctx.enter_context(tc.tile_pool(name="wpool", bufs=1))
            kpool = ctx.enter_context(tc.tile_pool(name="kpool", bufs=1))
            epool = ctx.enter_context(tc.tile_pool(name="epool", bufs=3))
            vpool = ctx.enter_context(tc.tile_pool(name="vpool", bufs=1))
            opool = ctx.enter_context(tc.tile_pool(name="opool", bufs=1))
            xpool = ctx.enter_context(tc.tile_pool(name="xpool", bufs=1))
            # PSUM: A = scores [128,2,512] x2 (4 banks; outproj at tail),
            #       B = pv x2 / kproj rounds x2 (2), C = rs (1), D = bc (1)
            ps = ctx.enter_context(tc.tile_pool(name="ps", bufs=1, space="PSUM"))

            ident = consts.tile([P, P], f32)
            make_identity(nc, ident[:])
            ones1x64f = consts.tile([1, 128], f32)
            nc.gpsimd.memset(ones1x64f[:], 1.0)
            ones1x128 = consts.tile([1, 128], f32r)
            nc.vector.tensor_copy(ones1x128[:], ones1x64f[:])
            ones_bf = consts.tile([P, 1], bf16)
            nc.gpsimd.memset(ones_bf[:], 1.0)

            wkT_bf = wpool.tile([P, NDC, D], bf16)       # [d, ei]
            woT = wpool.tile([P, NDC, D], bf16)          # [ei, eo]
            k_bf = kpool.tile([P, NSC, D], bf16)         # K [s, ei]
            khT = kpool.tile([P, NDC, S], bf16)          # K^T [ei, s]
            wvt = vpool.tile([P, NDC, SH], bf16)         # wV^T [ei, q_own]
            xT_bf = xpool.tile([P, NDC, S], bf16)        # x^T [d, s]

            # ---- prefix: wk -> wkT_bf;  x -> x_bf -> xT_bf (XBAR) ------
            with tc.tile_pool(name="stage", bufs=1) as stage:
                for dc2 in range(2):
                    wk_sb = stage.tile([P, NDC, 256], f32, tag="w", bufs=1,
                                       name="wk_sb")
                    nc.sync.dma_start(
                        wk_sb[:],
                        wk_d[:, dc2 * 256:(dc2 + 1) * 256].rearrange(
                            "(eo p) d -> p eo d", p=P))
                    spw = ps.tile([P, 2, D], f32, tag="A", bufs=2,
                                  name="spwk")
                    for dc in range(2):
                        for eo in range(NDC):
                            nc.tensor.transpose(
                                spw[:, dc, eo * P:(eo + 1) * P],
                                wk_sb[:, eo, dc * P:(dc + 1) * P], ident[:])
                    nc.scalar.copy(
                        wkT_bf[:, dc2 * 2:dc2 * 2 + 2, :], spw[:])

                for g in range(8):
                    x_g = stage.tile([P, 2, D], f32, tag="x", bufs=2,
                                     name="x_g")
                    nc.gpsimd.dma_start(
                        x_g[:],
                        x_d[g * 2 * P:(g + 1) * 2 * P, :].rearrange(
                            "(two p) d -> p two d", p=P))
                    x_bf = stage.tile([P, 2, D], bf16, tag="xbf", bufs=2,
                                      name="x_bf")
                    eng = nc.vector if g % 2 == 0 else nc.gpsimd
                    eng.tensor_copy(x_bf[:], x_g[:])
                    for i in range(2):
                        sc = g * 2 + i
                        nc.sync.dma_start_transpose(
                            xT_bf[:, :, sc * P:(sc + 1) * P], x_bf[:, i, :])

            def emit_kproj(ec):
                # khT[:, ec, :] = (Wk x^T) chunk: lhsT = wkT (stationary,
                # 2 matmuls per LDW), rhs = xT 512-col blocks.
                for sbp in range(2):
                    kps = ps.tile([P, 2, QB], f32, tag="B", bufs=1,
                                  name="kps")
                    for dc in range(NDC):
                        for j in range(2):
                            sb = sbp * 2 + j
                            nc.tensor.matmul(
                                kps[:, j, :],
                                wkT_bf[:, dc, ec * P:(ec + 1) * P],
                                xT_bf[:, dc, sb * QB:(sb + 1) * QB],
                                start=(dc == 0), stop=(dc == NDC - 1))
                    eng = nc.scalar if ec == 0 else nc.vector
                    if ec == 0:
                        eng.copy(
                            khT[:, ec, sbp * 2 * QB:(sbp * 2 + 2) * QB],
                            kps[:])
                    else:
                        eng.tensor_copy(
                            khT[:, ec, sbp * 2 * QB:(sbp * 2 + 2) * QB],
                            kps[:])
                nc.sync.dma_start_transpose(
                    k_bf[:, :, ec * P:(ec + 1) * P], khT[:, ec, :])

            def emit_wo():
                for dc2 in range(2):
                    wo_sb = wpool.tile([P, NDC, 256], f32, tag="wo_sb",
                                       bufs=2, name="wo_sb")
                    nc.gpsimd.dma_start(
                        wo_sb[:],
                        wo_d[:, dc2 * 256:(dc2 + 1) * 256].rearrange(
                            "(eo p) d -> p eo d", p=P))
                    spo = ps.tile([P, 2, D], f32, tag="A", bufs=2,
                                  name="spwo")
                    for dc in range(2):
                        for eo in range(NDC):
                            nc.tensor.transpose(
                                spo[:, dc, eo * P:(eo + 1) * P],
                                wo_sb[:, eo, dc * P:(dc + 1) * P],
                                ident[:])
                    nc.vector.tensor_copy(
                        woT[:, dc2 * 2:dc2 * 2 + 2, :], spo[:])

            emit_kproj(0)

            # ---- head-pair loop ----------------------------------------
            for pr in range(NPR):
                e_tiles = {}
                for qb in range(NQB):
                    e_tiles[qb] = epool.tile([P, 2, NSC, QB], bf16,
                                             name="e_t")
                for qb in range(NQB):
                    e_t = e_tiles[qb]
                    for kc in range(4 * qb, NSC):
                        sp = ps.tile([P, 2, QB], f32, tag="A", bufs=2,
                                     name="sp")
                        for hh in range(2):
                            nc.tensor.matmul(
                                sp[:, hh, :],
                                khT[hh * HD:(hh + 1) * HD, pr,
                                    kc * P:(kc + 1) * P],
                                khT[hh * HD:(hh + 1) * HD, pr,
                                    qb * QB:(qb + 1) * QB],
                                start=True, stop=True)
                        if "xhalf" in mode and kc % 2 == 1:
                            nc.vector.tensor_copy(
                                e_t[:, :, kc, 0:2], sp[:, :, 0:2])
                        else:
                            nc.scalar.activation(
                                e_t[:, :, kc, :], sp[:],
                                mybir.ActivationFunctionType.Exp,
                                scale=SCALE)
                        qd = kc // 4
                        if qb < qd < NQB:
                            qs = (kc - 4 * qd) * P
                            for hh in range(2):
                                nc.sync.dma_start_transpose(
                                    e_tiles[qd][:, hh, 4 * qb:4 * qb + 4,
                                                qs:qs + P],
                                    e_t[:, hh, kc, :])
                        if qb == 0 and kc == 7:
                            # hide the next chunk's kproj (and wo prep on
                            # pair 0) under this pair's exp stream
                            if pr < NPR - 1:
                                emit_kproj(pr + 1)
                            if pr == 0:
                                emit_wo()
                    # PV pair: col-tiled heads, accumulate over all 16 kc
                    pv = ps.tile([P, QB], f32, tag="B", bufs=1, name="pv")
                    for kc in range(NSC):
                        nc.tensor.matmul(
                            pv[0:HD, :],
                            k_bf[:, kc, pr * P:pr * P + HD],
                            e_t[:, 0, kc, :],
                            start=(kc == 0), stop=(kc == NSC - 1),
                            tile_position=(0, 0))
                        nc.tensor.matmul(
                            pv[HD:P, :],
                            k_bf[:, kc, pr * P + HD:pr * P + 2 * HD],
                            e_t[:, 1, kc, :],
                            start=(kc == 0), stop=(kc == NSC - 1),
                            skip_group_check=True,
                            tile_position=(0, HD))
                    nc.vector.tensor_copy(
                        wvt[:, pr, qb * QB:(qb + 1) * QB], pv[:])
                    # rowsum quad: (hh, kc-parity) 4-way col tiling
                    rs = ps.tile([97, QB], f32, tag="C", name="rs")
                    for kc in range(NSC):
                        for hh in range(2):
                            t = hh + 2 * (kc % 2)
                            nc.tensor.matmul(
                                rs[32 * t:32 * t + 1, :],
                                ones_bf[:, 0:1],
                                e_t[:, hh, kc, :],
                                start=(kc < 2),
                                stop=(kc >= NSC - 2),
                                skip_group_check=(t != 0),
                                tile_position=(0, 32 * t))
                    recips = []
                    for hh in range(2):
                        rse = vpool.tile([1, QB], f32, tag="rse", bufs=2,
                                         name="rse")
                        nc.vector.tensor_copy(rse[:],
                                              rs[32 * hh:32 * hh + 1, :])
                        rsum = vpool.tile([1, QB], f32, tag="rsum", bufs=2,
                                          name="rsum")
                        nc.vector.tensor_tensor(
                            rsum[:], rse[:],
                            rs[64 + 32 * hh:64 + 32 * hh + 1, :],
                            mybir.AluOpType.add)
                        recip = vpool.tile([1, QB], f32r, tag="recip",
                                           bufs=2, name="recip")
                        with nc.allow_low_precision(reason="recip fp32r ok"):
                            nc.vector.reciprocal(recip[:], rsum[:])
                        recips.append(recip)
                    for hh in range(2):
                        bc = ps.tile([P, QB], f32, tag="D", name="bc")
                        nc.tensor.matmul(bc[:], ones1x128[:],
                                         recips[hh][:],
                                         start=True, stop=True)
                        bc_sb = vpool.tile([P, QB], bf16, tag="bcsb",
                                           bufs=2, name="bc_sb")
                        nc.vector.tensor_copy(bc_sb[:], bc[:])
                        nc.vector.tensor_tensor(
                            wvt[hh * HD:(hh + 1) * HD, pr,
                                qb * QB:(qb + 1) * QB],
                            wvt[hh * HD:(hh + 1) * HD, pr,
                                qb * QB:(qb + 1) * QB],
                            bc_sb[hh * HD:(hh + 1) * HD, :],
                            mybir.AluOpType.mult)

            # ---- tail: out = wvt.T @ woT (local, 4 dc) -----------------
            for j4 in range(4):
                po = ps.tile([P, 2, QB], f32, tag="A", bufs=2, name="po")
                for j in range(2):
                    qc = j4 * 2 + j
                    for dc in range(NDC):
                        nc.tensor.matmul(
                            po[:, j, :],
                            wvt[:, dc, qc * P:(qc + 1) * P],
                            woT[:, dc, :],
                            start=(dc == 0), stop=(dc == NDC - 1))
                o_sb = opool.tile([P, 2, QB], f32, tag="osb", bufs=2,
                                  name="o_sb")
                nc.vector.tensor_copy(o_sb[:], po[:])
                qc0 = j4 * 2
                nc.gpsimd.dma_start(
                    out_d[qc0 * P:(qc0 + 2) * P, :].rearrange(
                        "(two p) d -> p two d", p=P),
                    o_sb[:])

    nc.compile()
    return nc


def _get_nc(repeat: int = 1, mode: str = "v4"):
    key = ("nc", repeat, mode)
    if key not in _CACHE:
        _CACHE[key] = _build_nc(repeat, mode)
    return _CACHE[key]


def make_in_maps(x, Wk, Wo):
    x = np.asarray(x, dtype=np.float32)
    Wk = np.ascontiguousarray(np.asarray(Wk, dtype=np.float32))
    Wo = np.ascontiguousarray(np.asarray(Wo, dtype=np.float32))
    in_maps = []
    for c in range(NCORES):
        b, half = c // 2, c % 2
        xb = x[b]
        if half:
            xb = np.roll(xb, -SH, axis=0)
        in_maps.append({"x": np.ascontiguousarray(xb), "Wk": Wk, "Wo": Wo})
    return in_maps


def kernel(x: np.ndarray, Wk: np.ndarray, Wo: np.ndarray, _trace=False):
    from concourse import bass_utils

    nc = _get_nc()
    in_maps = make_in_maps(x, Wk, Wo)
    res = bass_utils.run_bass_kernel_spmd(
        nc, in_maps, core_ids=list(range(NCORES)), trace=_trace)

    out = np.empty((B, S, D), dtype=np.float32)
    for c in range(NCORES):
        b, half = c // 2, c % 2
        out[b, half * SH:(half + 1) * SH] = res.results[c]["out"]
    if _trace:
        _CACHE["last_results"] = res
    return out
